# revision 10
# baseline (speedup 1.0000x reference)
"""Distributed causal attention kernel for one TRN2 chip (8 NeuronCores), v9.

Problem: B=4, T=2048, E=1024 single-head causal attention with QKV
projections (torch-Linear convention: y = x @ W.T + b).

Sharding (data-parallel + pair-wise K/V-projection dedup):
  8 cores = 4 batches x 2 query-groups. Core c handles batch b=c//2 and
  the 8 query tiles {2s+par : s=0..7} (par=c%2) of 128 rows each.
  Rank r=c%2 projects only global key quarters {r, r+2}; the pair
  exchanges projected kp/vp via four 1MB HBM->HBM pair AllGathers.

v9 (on top of the v8 schedule fixes):
  - FLAT 2D DMAs everywhere: the v7/v8 `rearrange("p (c t) -> p c t")`
    access patterns chopped every transfer into 1KB descriptors (biases:
    32B), capping each DMA queue at ~100GB/s.  All bulk tensors now move
    as [P, 4096]-style fully-contiguous blobs with 8KB partition lines.
  - Tensor order: warmup (release HAM clock gate) | K_p0 | V_p0 | K_p1 |
    V_p1 | Q | scores/softmax/AV, with each bounce write + pair
    AllGather trigger immediately after its producer, so the four
    gathers pipeline on the CC engine from ~30us and all four blob
    read-backs land well before their consumers.
  - kph0/1, vph0/1 get distinct SBUF buffers (no tile-reuse waits);
    wqh aliases wkq01's slots, qraw_p1 aliases kraw_p1 (dead by then)
    to fit the 208KB/partition SBUF budget.
  - rings: sync = raw k/v quarters + bounce writes; scalar = weights,
    qraw, mask, then all gather read-backs; gpsimd = bv/wv + the four
    AG triggers + output writes.
  - output scale split scalar/vector engines to shorten the tail.

Precision: all matmuls bf16 (fp8 re-measured: AV-only fp8 already 3.3%
rel err vs the 2e-2 gate -- rejected).  Scores computed transposed,
max-free softmax, denominator via a ones-column in the AV matmul.
"""

import math

import numpy as np
import ml_dtypes

import concourse.bass as bass
import concourse.tile as tile
from concourse import bacc, mybir
from concourse.bass_utils import run_bass_kernel_spmd

P = 128          # partition dim / tile unit
E = 1024         # n_embd
T = 2048         # sequence length
B = 4            # batch
OC = E // P      # 8 e/o chunks
S = 8            # query slots (128-row q tiles) per core
TC = T // P      # 16 key chunks
HALF = T // 2    # keys owned per core
QTR = 512        # key quarter
NEG = -1e9
BF = mybir.dt.bfloat16
F32 = mybir.dt.float32
SCALE = 1.0 / math.sqrt(E)
GROUPS = [[0, 1], [2, 3], [4, 5], [6, 7]]

# per key-chunk j: q columns [q0(j), 1024) participate
def _q0(j):
    return P * (j // 2)

_NQ = [S * P - _q0(j) for j in range(TC)]
_OFF = np.concatenate([[0], np.cumsum(_NQ)]).tolist()  # probsT column offsets
_PROBS_COLS = int(_OFF[-1])  # 9216


def _subchunks(n, step=512):
    out = []
    c = 0
    while c < n:
        out.append((c, min(step, n - c)))
        c += step
    return out


def _flat(t3):
    """[P, C, N] tile AP -> contiguous [P, C*N] view (8KB DMA lines)."""
    return t3.rearrange("p c n -> p (c n)")


def build_nc():
    nc = bacc.Bacc("TRN2", target_bir_lowering=False, debug=False, num_devices=8)

    # [P, *] C-order blob parameters; piece-blobs packed side by side
    qTr = nc.declare_dram_parameter("qTr", [P, OC * S * P], BF, isOutput=False)
    kTr = nc.declare_dram_parameter("kTr", [P, OC * HALF], BF, isOutput=False)
    vTr = nc.declare_dram_parameter("vTr", [P, OC * HALF], BF, isOutput=False)
    wqr = nc.declare_dram_parameter("wqr", [P, OC * E], BF, isOutput=False)
    wkr = nc.declare_dram_parameter("wkr", [P, OC * E], BF, isOutput=False)
    wvr = nc.declare_dram_parameter("wvr", [P, OC * E], BF, isOutput=False)
    bqr = nc.declare_dram_parameter("bqr", [P, OC], F32, isOutput=False)
    bkr = nc.declare_dram_parameter("bkr", [P, OC], F32, isOutput=False)
    bvr = nc.declare_dram_parameter("bvr", [1, E], F32, isOutput=False)
    maskT = nc.declare_dram_parameter("maskT", [P, 2 * P], F32, isOutput=False)
    out_ext = nc.declare_dram_parameter("out", [S * P, E], BF, isOutput=True)

    MB = OC * QTR  # 4096 cols = one 1MB piece-blob

    with tile.TileContext(nc) as tc:
        with (
            tc.tile_pool(name="singles", bufs=1) as singles,
            tc.tile_pool(name="stream", bufs=2) as stream,
            tc.tile_pool(name="stage", bufs=1) as stage,
            tc.tile_pool(name="outp", bufs=2) as outp,
            tc.tile_pool(name="dram", bufs=1, space="DRAM") as dram,
            tc.tile_pool(name="mmps", bufs=2, space="PSUM") as mmps,
            tc.tile_pool(name="avps", bufs=1, space="PSUM") as avps,
        ):
            dma = nc.sync
            dma3 = nc.scalar
            dmag = nc.gpsimd

            # ---------- DRAM bounce + gathered blobs (raw SBUF order) ----------
            kph_d = [dram.tile([P, MB], BF, name=f"kph_d{i}") for i in range(2)]
            vph_d = [dram.tile([P, 4 * E], BF, name=f"vph_d{i}") for i in range(2)]
            kpg_d = [dram.tile([2 * P, MB], BF, name=f"kpg_d{i}") for i in range(2)]
            vpg_d = [dram.tile([2 * P, 4 * E], BF, name=f"vpg_d{i}") for i in range(2)]

            # ---------- resident tiles ----------
            wkq_sb = [
                singles.tile([P, OC, 2 * P], BF, tag=f"wk{i}", name=f"wkq_sb{i}")
                for i in range(4)
            ]
            wvh_sb = [
                singles.tile([P, OC, QTR], BF, name=f"wvh_sb{i}") for i in range(2)
            ]
            bq_sb = singles.tile([P, OC], F32)
            bk_sb = singles.tile([P, OC], F32)
            bv_sb = singles.tile([P, E], F32)
            mask_sb = singles.tile([P, 2 * P], F32)
            qpT = singles.tile([P, OC, S * P], BF)   # [p, o-chunk, q]
            vp = singles.tile([P, TC, E], BF)        # [p, t2-chunk, e]
            probsT = singles.tile([P, _PROBS_COLS], BF)
            recip_sb = singles.tile([P, S], F32)
            ones_sb = singles.tile([P, P], BF)

            kraw_p = [
                stage.tile([P, OC, QTR], BF, tag=f"kraw{i}", name=f"kraw_p{i}")
                for i in range(2)
            ]
            vraw_p = [
                stage.tile([P, OC, QTR], BF, tag=f"vraw{i}", name=f"vraw_p{i}")
                for i in range(2)
            ]

            def blob_ap(src, i, cols=MB):
                return src.ap()[:, cols * i : cols * (i + 1)]

            # sync ring: raw k/v quarters in first-use order; the kraw_p0
            # halves land separately so K_p0's e-accumulation can stream.
            dma.dma_start(out=_flat(kraw_p[0][:, 0:4, :]), in_=blob_ap(kTr, 0, 2048))
            dma.dma_start(out=_flat(kraw_p[0][:, 4:8, :]), in_=blob_ap(kTr, 1, 2048))
            dma.dma_start(out=_flat(vraw_p[0]), in_=blob_ap(vTr, 0))
            dma.dma_start(out=_flat(kraw_p[1]), in_=blob_ap(kTr, 1))
            dma.dma_start(out=_flat(vraw_p[1]), in_=blob_ap(vTr, 1))
            # scalar ring: wk quarters + small consts (weights lead; the
            # first matmul needs wkq0 + the first kraw half)
            dma3.dma_start(out=_flat(wkq_sb[0]), in_=blob_ap(wkr, 0, 2048))
            dma3.dma_start(out=bk_sb, in_=bkr.ap())
            dma3.dma_start(out=_flat(wkq_sb[1]), in_=blob_ap(wkr, 1, 2048))
            dma3.dma_start(out=_flat(wkq_sb[2]), in_=blob_ap(wkr, 2, 2048))
            dma3.dma_start(out=_flat(wkq_sb[3]), in_=blob_ap(wkr, 3, 2048))
            dma3.dma_start(out=bq_sb, in_=bqr.ap())
            # scalar ring continues: wv halves + bv broadcast (gpsimd must
            # stay empty ahead of the AllGather triggers)
            for i in range(2):
                dma3.dma_start(out=_flat(wvh_sb[i]), in_=blob_ap(wvr, i))
            bv_ap = bvr.ap()
            dma3.dma_start(
                out=bv_sb,
                in_=bass.AP(
                    tensor=bv_ap.tensor, offset=bv_ap.offset, ap=[[0, P], [1, E]]
                ),
            )

            # ---------- PE warmup: release the HAM clock gate while the
            # first weight/kraw DMAs land (cold MMs ~107ns)
            nc.vector.memset(ones_sb, 1.0)
            wup = mmps.tile([P, P], F32, tag="mm")
            for _ in range(150):
                nc.tensor.matmul(
                    wup, lhsT=ones_sb, rhs=ones_sb, start=True, stop=True
                )

            # ---------- interleaved K/V half-projections: K_p0 V_p0 K_p1
            # V_p1, each followed by its bounce write + AllGather trigger
            kph_sb = {}
            vph_sb = {}

            def kproj(piece):
                kph_sb[piece] = stage.tile(
                    [P, OC, QTR], BF, tag=f"kph{piece}", name=f"kph{piece}"
                )
                for o in range(OC):
                    acc = mmps.tile([P, QTR], F32, tag="mm")
                    for e in range(OC):
                        nc.tensor.matmul(
                            acc,
                            lhsT=wkq_sb[o // 2][:, e, (o % 2) * P : (o % 2 + 1) * P],
                            rhs=kraw_p[piece][:, e, :],
                            start=(e == 0),
                            stop=(e == OC - 1),
                        )
                    nc.vector.tensor_scalar(
                        out=kph_sb[piece][:, o, :],
                        in0=acc,
                        scalar1=bk_sb[:, o : o + 1],
                        scalar2=None,
                        op0=mybir.AluOpType.add,
                    )
                dma.dma_start(out=kph_d[piece][:], in_=_flat(kph_sb[piece]))
                nc.gpsimd.collective_compute(
                    "AllGather",
                    mybir.AluOpType.bypass,
                    replica_groups=GROUPS,
                    ins=[kph_d[piece].opt()],
                    outs=[kpg_d[piece].opt()],
                )

            def vproj(piece):
                vph_sb[piece] = stage.tile(
                    [P, 4, E], BF, tag=f"vph{piece}", name=f"vph{piece}"
                )
                for eh in range(2):
                    for jl in range(4):
                        acc = mmps.tile([P, QTR], F32, tag="mm")
                        for e in range(OC):
                            nc.tensor.matmul(
                                acc,
                                lhsT=vraw_p[piece][:, e, jl * P : (jl + 1) * P],
                                rhs=wvh_sb[eh][:, e, :],
                                start=(e == 0),
                                stop=(e == OC - 1),
                            )
                        nc.vector.tensor_add(
                            out=vph_sb[piece][:, jl, 512 * eh : 512 * (eh + 1)],
                            in0=acc,
                            in1=bv_sb[:, 512 * eh : 512 * (eh + 1)],
                        )
                dma.dma_start(out=vph_d[piece][:], in_=_flat(vph_sb[piece]))
                nc.gpsimd.collective_compute(
                    "AllGather",
                    mybir.AluOpType.bypass,
                    replica_groups=GROUPS,
                    ins=[vph_d[piece].opt()],
                    outs=[vpg_d[piece].opt()],
                )

            kproj(0)
            vproj(0)
            kproj(1)
            vproj(1)

            # scalar ring continues: wq halves (alias wkq01 slots), qraw
            # quarters (qraw_p1 aliases kraw_p1), mask, then ALL gather
            # read-backs in consumption order.
            wqh_sb = [
                singles.tile(
                    [P, OC, QTR], BF, tag=f"wk{i}", name=f"wqh_sb{i}"
                )
                for i in range(2)
            ]
            for i in range(2):
                dma3.dma_start(out=_flat(wqh_sb[i]), in_=blob_ap(wqr, i))
            qraw_p = [
                stage.tile([P, OC, QTR], BF, tag="qraw0", name="qraw_p0"),
                stage.tile([P, OC, QTR], BF, tag="kraw1", name="qraw_p1"),
            ]
            for i in range(2):
                dma3.dma_start(out=_flat(qraw_p[i]), in_=blob_ap(qTr, i))
            dma3.dma_start(out=mask_sb, in_=maskT.ap())

            # gather read-backs (scalar ring): kpq0, kpq1, vp0, vp1, kpq2,
            # kpq3, vp2, vp3 -- ordered by first consumer.
            kpq_tiles = {}

            def load_kpq(kq):
                g, h = kq // 2, kq % 2
                t = stream.tile([P, OC, QTR], BF, tag="kpq", name=f"kpq{kq}")
                dma3.dma_start(out=_flat(t), in_=kpg_d[g][P * h : P * (h + 1), :])
                kpq_tiles[kq] = t

            def load_vp(kq):
                g, h = kq // 2, kq % 2
                dma3.dma_start(
                    out=_flat(vp[:, 4 * kq : 4 * kq + 4, :]),
                    in_=vpg_d[g][P * h : P * (h + 1), :],
                )

            load_kpq(0)
            load_kpq(1)
            load_vp(0)
            load_vp(1)
            load_kpq(2)
            load_kpq(3)
            load_vp(2)
            load_vp(3)

            # ---------- Q projection ----------
            for qq in range(2):
                for o in range(OC):
                    acc = mmps.tile([P, QTR], F32, tag="mm")
                    for e in range(OC):
                        nc.tensor.matmul(
                            acc,
                            lhsT=wqh_sb[o // 4][:, e, (o % 4) * P : (o % 4 + 1) * P],
                            rhs=qraw_p[qq][:, e, :],
                            start=(e == 0),
                            stop=(e == OC - 1),
                        )
                    nc.vector.tensor_scalar(
                        out=qpT[:, o, QTR * qq : QTR * (qq + 1)],
                        in0=acc,
                        scalar1=bq_sb[:, o : o + 1],
                        scalar2=None,
                        op0=mybir.AluOpType.add,
                    )

            # ---------- attention: scores / softmax / AV ----------
            for j in range(TC):
                # scoresT chunk j: [t2=128, q=Nq]
                q0 = _q0(j)
                nq = _NQ[j]
                st = mmps.tile([P, nq], F32, tag="mm")
                kpq = kpq_tiles[j // 4]
                jq = j % 4
                for o in range(OC):
                    for c0, cw in _subchunks(nq):
                        nc.tensor.matmul(
                            st[:, c0 : c0 + cw],
                            lhsT=kpq[:, o, jq * P : (jq + 1) * P],
                            rhs=qpT[:, o, q0 + c0 : q0 + c0 + cw],
                            start=(o == 0),
                            stop=(o == OC - 1),
                        )
                # causal mask on the first 128 q columns (slot j//2)
                nc.vector.tensor_add(
                    out=st[:, 0:P],
                    in0=st[:, 0:P],
                    in1=mask_sb[:, (j % 2) * P : (j % 2 + 1) * P],
                )
                # probsT = exp(scoresT / sqrt(E))
                nc.scalar.activation(
                    out=probsT[:, _OFF[j] : _OFF[j] + nq],
                    in_=st,
                    func=mybir.ActivationFunctionType.Exp,
                    scale=SCALE,
                )

                # AV for slot s = (j-1)//2 once its last chunk (j=2s+1) is done
                if j % 2 == 1:
                    s = j // 2
                    nchunks = j + 1
                    av = avps.tile([P, 1536], F32, tag="av")
                    for jj in range(nchunks):
                        lhsT = probsT[
                            :,
                            _OFF[jj]
                            + (s - jj // 2) * P : _OFF[jj]
                            + (s - jj // 2) * P
                            + P,
                        ]
                        st_f = jj == 0
                        sp_f = jj == nchunks - 1
                        nc.tensor.matmul(
                            av[:, 1024:1025],
                            lhsT=lhsT,
                            rhs=ones_sb[:, 0:1],
                            start=st_f,
                            stop=sp_f,
                        )
                        for eh in range(2):
                            nc.tensor.matmul(
                                av[:, 512 * eh : 512 * (eh + 1)],
                                lhsT=lhsT,
                                rhs=vp[:, jj, 512 * eh : 512 * (eh + 1)],
                                start=st_f,
                                stop=sp_f,
                            )
                    nc.vector.reciprocal(
                        out=recip_sb[:, s : s + 1], in_=av[:, 1024:1025]
                    )
                    osb = outp.tile([P, E], BF, tag="osb")
                    nc.scalar.mul(
                        out=osb[:, 0:512],
                        in_=av[:, 0:512],
                        mul=recip_sb[:, s : s + 1],
                    )
                    nc.vector.tensor_scalar(
                        out=osb[:, 512:1024],
                        in0=av[:, 512:1024],
                        scalar1=recip_sb[:, s : s + 1],
                        scalar2=None,
                        op0=mybir.AluOpType.mult,
                    )
                    dmag.dma_start(
                        out=out_ext.ap()[P * s : P * (s + 1), :],
                        in_=osb,
                    )

    nc.finalize()
    return nc


_NC_CACHE = {}


def _get_nc():
    if "nc" not in _NC_CACHE:
        _NC_CACHE["nc"] = build_nc()
    return _NC_CACHE["nc"]


def _bf16(x):
    return np.asarray(x, np.float32).astype(ml_dtypes.bfloat16)


def _blob(a2d):
    """[C*P, N] -> [P, C*N] prearranged C-order blob (SBUF layout)."""
    cp, n = a2d.shape
    c = cp // P
    return np.ascontiguousarray(
        a2d.reshape(c, P, n).transpose(1, 0, 2).reshape(P, c * n)
    )


def _blobc(a2d, npiece):
    """[C*P, M] -> [P, C*M]: npiece side-by-side col-split piece-blobs."""
    n = a2d.shape[1] // npiece
    return np.ascontiguousarray(
        np.hstack([_blob(a2d[:, n * i : n * (i + 1)]) for i in range(npiece)])
    )


def make_in_maps(q, k, v, wq_w, wq_b, wk_w, wk_b, wv_w, wv_b):
    """Host-side sharding: returns list of 8 per-core input dicts."""
    q = np.asarray(q, np.float32)
    k = np.asarray(k, np.float32)
    v = np.asarray(v, np.float32)
    wqr = _blobc(_bf16(np.asarray(wq_w).T), 2)
    wkr = _blobc(_bf16(np.asarray(wk_w).T), 4)
    wvr = _blobc(_bf16(np.asarray(wv_w).T), 2)
    bqr = np.ascontiguousarray(np.asarray(wq_b, np.float32).reshape(OC, P).T)
    bkr = np.ascontiguousarray(np.asarray(wk_b, np.float32).reshape(OC, P).T)
    bvr = np.asarray(wv_b, np.float32).reshape(1, E)

    r_ = np.arange(P)
    tril = np.where(r_[:, None] <= r_[None, :], 0.0, NEG).astype(np.float32)
    mask_even = np.concatenate([tril, np.full((P, P), NEG, np.float32)], axis=1)
    mask_odd = np.concatenate([np.zeros((P, P), np.float32), tril], axis=1)

    in_maps = []
    for c in range(8):
        b, par = c // 2, c % 2
        rows = np.concatenate(
            [np.arange(P * (2 * s + par), P * (2 * s + par) + P) for s in range(S)]
        )
        # rank par owns global key quarters {par, par+2}
        kcols = np.concatenate(
            [np.arange(QTR * par, QTR * par + QTR),
             np.arange(QTR * (par + 2), QTR * (par + 2) + QTR)]
        )
        kT = _bf16(k[b]).T
        vT = _bf16(v[b]).T
        in_maps.append(
            {
                "qTr": _blobc(_bf16(q[b][rows]).T, 2),
                "kTr": _blobc(kT[:, kcols], 2),
                "vTr": _blobc(vT[:, kcols], 2),
                "wqr": wqr,
                "wkr": wkr,
                "wvr": wvr,
                "bqr": bqr,
                "bkr": bkr,
                "bvr": bvr,
                "maskT": mask_even if par == 0 else mask_odd,
            }
        )
    return in_maps


def assemble_out(per_core_outs):
    """Inverse of the query sharding: returns [B, T, E] f32."""
    out = np.empty((B, T, E), np.float32)
    for c in range(8):
        b, par = c // 2, c % 2
        o = np.asarray(per_core_outs[c], dtype=np.float32)
        for s in range(S):
            out[b, P * (2 * s + par) : P * (2 * s + par) + P, :] = o[
                P * s : P * (s + 1), :
            ]
    return out


def _kernel_np_fallback(q, k, v, wq_w, wq_b, wk_w, wk_b, wv_w, wv_b, causal):
    """Numpy reference path (used only for the causal=0 edge case)."""
    q = np.asarray(q, np.float32)
    out = np.empty_like(q)
    for b in range(q.shape[0]):
        qp = q[b] @ np.asarray(wq_w, np.float32).T + np.asarray(wq_b, np.float32)
        kp = np.asarray(k[b], np.float32) @ np.asarray(wk_w, np.float32).T + np.asarray(
            wk_b, np.float32
        )
        vp = np.asarray(v[b], np.float32) @ np.asarray(wv_w, np.float32).T + np.asarray(
            wv_b, np.float32
        )
        s = (qp @ kp.T) * SCALE
        if causal:
            t = s.shape[0]
            s = np.where(np.tril(np.ones((t, t), bool)), s, -np.inf)
        s -= s.max(-1, keepdims=True)
        p = np.exp(s)
        out[b] = (p @ vp) / p.sum(-1, keepdims=True)
    return out


def kernel(q, k, v, wq_w, wq_b, wk_w, wk_b, wv_w, wv_b, causal, **run_kwargs):
    if not int(causal):
        return _kernel_np_fallback(
            q, k, v, wq_w, wq_b, wk_w, wk_b, wv_w, wv_b, causal
        )
    nc = _get_nc()
    in_maps = make_in_maps(q, k, v, wq_w, wq_b, wk_w, wk_b, wv_w, wv_b)
    res = run_bass_kernel_spmd(nc, in_maps, core_ids=list(range(8)), **run_kwargs)
    out = assemble_out([r["out"] for r in res.results])
    if run_kwargs:
        kernel.last_results = res
    return out
